# revision 10
# baseline (speedup 1.0000x reference)
"""Ewald realspace (N=3072) on 8 Trainium2 NeuronCores.

Row-sharded: core c owns rows i in [c*384, (c+1)*384) (in a permuted atom
order). Each core computes, for every column j, partial contractions
sum_i W[i,j] * V[i,col] for three pairwise kernels W (e1 = erf*rinv, s1,
SS = s2*rinv2) against a 21-column matrix V of per-atom quantities. The host
combines the 8 partial results into e_phi / e_field / pot exactly as the
reference does.

Device math per 128x512 block (i on partitions, j on free dim):
  d2  = |r_i|^2 + |r_j|^2 - 2 r_i.r_j   (plain-fp32 PE matmul, K=5,
        box-centered coordinates to minimize cancellation error)
        + 2^40 on the diagonal AND on close pairs (DVE add of a host mask)
  rnorm  = sqrt(d2)            [ACT Sqrt;   sqrt set]
  d2sb   = copy(d2)            [ACT Copy;   any set]
  gaussC = exp(-d2/2 + ln(2A/sqrt(pi)))  [ACT Exp;  exp set; 2-ULP table]
  erf_t  = erf(A*rnorm)        [ACT Erf;    sigmoid set]
  rinv   = recip_approx_fast(rnorm)   [DVE]
  rinv2  = Square(rinv)        [ACT Square; any set]
  e1 = erf_t*rinv;  t2 = e1 - gaussC;  s1 = t2*rinv2
  s2 = 3*s1 - gaussC  (pairwise - the cancellation is catastrophic if
                       deferred past the contraction);  SS = s2*rinv2
All three streams are contracted in plain fp32 (PE 2-pass matmuls) with fp32
PSUM accumulation over the core's 3 row-tiles.

Close pairs (d2 < CLOSE_CUT) break the s2 double-cancellation under any
device noise, and the r_j - r_i contraction decomposition amplifies their
error by |r_j|/|r_ij|. So: atoms are permuted so every close pair lands in
one 128-row tile, those pairs are padded out on-device (d2 += 2^40 via the
mask), and the host adds their exact fp64 contributions.

ACT table sets are phase-grouped (sqrt -> exp -> sigmoid) with explicit
scheduling deps so each set loads exactly once per kernel.

Columns are rotated per-core by -c*384 so the diagonal block always lands in
j-block 0 at offset 128*it - keeping the program SPMD-identical across cores.
"""

import time
import numpy as np
from contextlib import ExitStack

import concourse.bass as bass
import concourse.tile as tile
from concourse import bacc, mybir
from concourse.bass_utils import run_bass_kernel_spmd

N = 3072
N_CORES = 8
ROWS = N // N_CORES          # 384
ITILES = ROWS // 128         # 3
JBLK = 512
NJB = N // JBLK              # 6
NCOL = 21                    # V columns actually used
NSTREAM = 3                  # e1, s1, SS

SIGMA = 1.0
NORM_FACTOR = 90.4756
NORM_CONST = NORM_FACTOR / (2.0 * np.pi)
A = 1.0 / (SIGMA * 2.0 ** 0.5)
LN_C2P = float(np.log(2.0 * A / np.sqrt(np.pi)))   # exp bias -> C2P*gauss
BIG = 2.0 ** 40              # d2 pad for diagonal + close pairs
DIAG_E1 = 2.0 ** -20         # e1 value on padded entries (1/sqrt(BIG))
CLOSE_CUT = 2.0              # d2 threshold for host-corrected close pairs

F32 = mybir.dt.float32
AF = mybir.ActivationFunctionType

_CACHE = {}


def _build():
    from concourse.tile_rust import add_dep_helper

    nc = bacc.Bacc("TRN2", target_bir_lowering=False, debug=False,
                   num_devices=N_CORES)
    lhs5_ap = nc.dram_tensor("lhs5", [5, ROWS], F32, kind="ExternalInput").ap()
    rhs5_ap = nc.dram_tensor("rhs5", [5, N], F32, kind="ExternalInput").ap()
    vall_ap = nc.dram_tensor("vall", [128, ITILES * 32], F32,
                             kind="ExternalInput").ap()
    eyek_ap = nc.dram_tensor("eyek", [128, ROWS], F32,
                             kind="ExternalInput").ap()
    out_ap = nc.dram_tensor("out", [NJB, NSTREAM, NCOL, JBLK], F32,
                            kind="ExternalOutput").ap()

    with tile.TileContext(nc) as tc:
        with ExitStack() as ctx:
            const = ctx.enter_context(tc.tile_pool(name="const", bufs=1))
            psum_d2 = ctx.enter_context(
                tc.tile_pool(name="psum_d2", bufs=3, space="PSUM"))
            psum_out = ctx.enter_context(
                tc.tile_pool(name="psum_out", bufs=1, space="PSUM"))
            rnorm_p = ctx.enter_context(tc.tile_pool(name="rnorm", bufs=18))
            d2sb_p = ctx.enter_context(tc.tile_pool(name="d2sb", bufs=18))
            gauss_p = ctx.enter_context(tc.tile_pool(name="gauss", bufs=18))
            trans = ctx.enter_context(tc.tile_pool(name="trans", bufs=3))
            outsb_p = ctx.enter_context(tc.tile_pool(name="outsb", bufs=2))

            lhs5 = const.tile([5, ROWS], F32)
            nc.sync.dma_start(lhs5[:], lhs5_ap[:])
            rhs5 = const.tile([5, N], F32)
            nc.sync.dma_start(rhs5[:], rhs5_ap[:])
            vall_f = const.tile([128, ITILES * 32], F32)
            nc.sync.dma_start(vall_f[:], vall_ap[:])
            eyek = const.tile([128, ROWS], F32)
            nc.sync.dma_start(eyek[:], eyek_ap[:])
            # per-partition bias constant for the Exp activation
            expbias = const.tile([128, 1], F32)
            nc.vector.memset(expbias[:], LN_C2P)

            nb = NJB * ITILES
            rnorm = [None] * nb
            d2sb = [None] * nb
            gauss = [None] * nb

            # ------------- P1: d2 matmuls + Sqrt + Copy (sqrt set) ---------
            last_act = None
            for jb in range(NJB):
                for it in range(ITILES):
                    b = jb * ITILES + it
                    d2 = psum_d2.tile([128, JBLK], F32, tag="d2")
                    nc.tensor.matmul(
                        d2[:],
                        lhs5[:, it * 128:(it + 1) * 128],
                        rhs5[:, jb * JBLK:(jb + 1) * JBLK],
                        start=True, stop=True)
                    if jb == 0:
                        off = it * 128
                        nc.vector.tensor_add(
                            d2[:, off:off + 128], d2[:, off:off + 128],
                            eyek[:, off:off + 128])
                    rn = rnorm_p.tile([128, JBLK], F32, tag="rnorm")
                    nc.scalar.activation(rn[:], d2[:], AF.Sqrt)
                    ds = d2sb_p.tile([128, JBLK], F32, tag="d2sb")
                    i1b = nc.scalar.copy(ds[:], d2[:])
                    last_act = i1b
                    rnorm[b] = rn
                    d2sb[b] = ds

            # ------------- P2: Exp (exp set) -------------------------------
            p1_last = last_act
            for b in range(nb):
                ga = gauss_p.tile([128, JBLK], F32, tag="gauss")
                i2 = nc.scalar.activation(ga[:], d2sb[b][:], AF.Exp,
                                          scale=-0.5, bias=expbias[:])
                add_dep_helper(i2.ins, p1_last.ins, sync=False,
                               reason="ACT phase order: exp after all sqrt")
                last_act = i2
                gauss[b] = ga

            # ------- P3: Erf + Square (sigmoid set) + DVE chain + PE -------
            p2_last = last_act
            for jb in range(NJB):
                outs = [psum_out.tile([NCOL, JBLK], F32, tag=f"out{w}",
                                      name=f"out{w}_jb{jb}")
                        for w in range(NSTREAM)]
                for it in range(ITILES):
                    b = jb * ITILES + it
                    erf_t = trans.tile([128, JBLK], F32, tag="erf")
                    i3 = nc.scalar.activation(erf_t[:], rnorm[b][:], AF.Erf,
                                              scale=float(A))
                    add_dep_helper(i3.ins, p2_last.ins, sync=False,
                                   reason="ACT phase order: erf after all exp")
                    rinv = trans.tile([128, JBLK], F32, tag="rinv")
                    nc.vector.reciprocal_approx_fast(rinv[:], rnorm[b][:])
                    rinv2 = trans.tile([128, JBLK], F32, tag="rinv2")
                    i4 = nc.scalar.activation(rinv2[:], rinv[:], AF.Square)
                    add_dep_helper(i4.ins, p2_last.ins, sync=False,
                                   reason="ACT phase order: square after exp")

                    e1 = trans.tile([128, JBLK], F32, tag="e1")
                    nc.vector.tensor_mul(e1[:], erf_t[:], rinv[:])
                    t2 = trans.tile([128, JBLK], F32, tag="t2")
                    nc.vector.tensor_sub(t2[:], e1[:], gauss[b][:])
                    s1 = trans.tile([128, JBLK], F32, tag="s1")
                    nc.vector.tensor_mul(s1[:], t2[:], rinv2[:])
                    s3 = trans.tile([128, JBLK], F32, tag="s3")
                    nc.vector.tensor_scalar_mul(s3[:], s1[:], 3.0)
                    s2 = trans.tile([128, JBLK], F32, tag="s2")
                    nc.vector.tensor_sub(s2[:], s3[:], gauss[b][:])
                    ss = trans.tile([128, JBLK], F32, tag="ss")
                    nc.vector.tensor_mul(ss[:], s2[:], rinv2[:])

                    for w, st in ((0, e1), (1, s1), (2, ss)):
                        nc.tensor.matmul(
                            outs[w][:],
                            vall_f[:, it * 32: it * 32 + NCOL],
                            st[:],
                            start=(it == 0), stop=(it == ITILES - 1))

                for w in range(NSTREAM):
                    osb = outsb_p.tile([NCOL, JBLK], F32, tag=f"osb{w}",
                                       name=f"osb{w}_jb{jb}")
                    nc.scalar.copy(osb[:], outs[w][:])
                    nc.sync.dma_start(out_ap[jb, w], osb[:])

    nc.compile()
    return nc


def _close_pair_permutation(r64):
    """Find close pairs (d2 < CLOSE_CUT) and a permutation packing each
    connected component of the close-pair graph into one 128-atom tile.

    Returns (perm, pairs) where pairs are ordered (i, j) index pairs IN THE
    PERMUTED numbering, i != j, covering both directions of every close pair
    that ended up intra-tile."""
    d2 = ((r64[:, None, :] - r64[None, :, :]) ** 2).sum(-1)
    np.fill_diagonal(d2, np.inf)
    ii, jj = np.nonzero(d2 < CLOSE_CUT)
    # union-find over close-pair graph
    parent = np.arange(N)

    def find(x):
        while parent[x] != x:
            parent[x] = parent[parent[x]]
            x = parent[x]
        return x

    for i, j in zip(ii, jj):
        ri, rj = find(i), find(j)
        if ri != rj:
            parent[ri] = rj
    comps = {}
    for i in range(N):
        comps.setdefault(find(i), []).append(i)
    groups = sorted(comps.values(), key=len, reverse=True)
    # first-fit pack into 24 tiles of 128
    ntiles = N // 128
    bins = [[] for _ in range(ntiles)]
    leftovers = []
    for g in groups:
        if len(g) > 128:
            leftovers.extend(g)     # unpackable; pairs inside stay uncorrected
            continue
        placed = False
        for b in bins:
            if len(b) + len(g) <= 128:
                b.extend(g)
                placed = True
                break
        if not placed:
            leftovers.extend(g)
    for x in leftovers:
        for b in bins:
            if len(b) < 128:
                b.append(x)
                break
    perm = np.array([x for b in bins for x in b])
    assert len(perm) == N and len(set(perm.tolist())) == N
    inv = np.empty(N, np.int64)
    inv[perm] = np.arange(N)
    # close pairs in permuted numbering, keep only intra-tile ones
    pi, pj = inv[ii], inv[jj]
    intra = (pi // 128) == (pj // 128)
    pairs = np.stack([pi[intra], pj[intra]], 1)
    return perm, pairs


def _prep_inputs(q, r, u):
    """Permute atoms, center coordinates, build per-core input maps.

    Returns (in_maps, V, perm, pairs, rc64) - everything in PERMUTED space."""
    r64g = r.astype(np.float64)
    perm, pairs = _close_pair_permutation(r64g)
    center = (r64g.max(0) + r64g.min(0)) / 2.0
    rc64 = (r64g - center)[perm]
    q64 = q.astype(np.float64)[perm]
    u64 = u.astype(np.float64)[perm]
    rr = (rc64 ** 2).sum(1)
    w = (u64 * rc64).sum(1)

    rhs5_g = np.stack([rc64[:, 0], rc64[:, 1], rc64[:, 2],
                       np.ones(N), rr]).astype(np.float32)        # [5, N]
    lhs5_g = np.stack([-2 * rc64[:, 0], -2 * rc64[:, 1], -2 * rc64[:, 2],
                       rr, np.ones(N)]).astype(np.float32)        # [5, N]

    V = np.zeros((N, NCOL), np.float64)
    V[:, 0] = q64
    V[:, 1:4] = u64
    V[:, 4] = w
    V[:, 5:8] = q64[:, None] * rc64
    for c in range(3):
        for d in range(3):
            V[:, 8 + 3 * c + d] = u64[:, c] * rc64[:, d]
    V[:, 17:20] = w[:, None] * rc64
    V[:, 20] = 1.0
    Vf = V.astype(np.float32)

    # pad mask per tile: diagonal + intra-tile close pairs
    eyek_g = np.zeros((N // 128, 128, 128), np.float32)
    for t in range(N // 128):
        eyek_g[t][np.arange(128), np.arange(128)] = BIG
    for i, j in pairs:
        t = i // 128
        eyek_g[t][i % 128, j % 128] = BIG

    in_maps = []
    for c in range(N_CORES):
        sl = slice(c * ROWS, (c + 1) * ROWS)
        vall = np.zeros((128, ITILES * 32), np.float32)
        eyek = np.zeros((128, ROWS), np.float32)
        for it in range(ITILES):
            gt = c * ITILES + it
            rows = slice(gt * 128, (gt + 1) * 128)
            vall[:, it * 32: it * 32 + NCOL] = Vf[rows]
            eyek[:, it * 128:(it + 1) * 128] = eyek_g[gt]
        in_maps.append({
            "lhs5": np.ascontiguousarray(lhs5_g[:, sl]),
            "rhs5": np.ascontiguousarray(np.roll(rhs5_g, -c * ROWS, axis=1)),
            "vall": vall,
            "eyek": eyek,
        })
    return in_maps, V, perm, pairs, rc64


def _host_corrections(G, V, pairs, rc64):
    """Remove padded-entry spurious e1 and add exact fp64 close-pair terms."""
    # spurious e1 = DIAG_E1 on the diagonal and every padded pair
    G[0] -= V.T * DIAG_E1
    if len(pairs):
        np.subtract.at(G[0].T, pairs[:, 1], V[pairs[:, 0]] * DIAG_E1)
        d2c = ((rc64[pairs[:, 0]] - rc64[pairs[:, 1]]) ** 2).sum(1)
        rn = np.sqrt(d2c)
        ri = 1.0 / rn
        ri2 = ri * ri
        from scipy.special import erf as serf
        ef = serf(A * rn)
        ga = (2 * A / np.sqrt(np.pi)) * np.exp(-0.5 * d2c)
        e1c = ef * ri
        s1c = (e1c - ga) * ri2
        SSc = (3 * s1c - ga) * ri2
        np.add.at(G[0].T, pairs[:, 1], V[pairs[:, 0]] * e1c[:, None])
        np.add.at(G[1].T, pairs[:, 1], V[pairs[:, 0]] * s1c[:, None])
        np.add.at(G[2].T, pairs[:, 1], V[pairs[:, 0]] * SSc[:, None])
    return G


def _combine(G, q64, rc64, u64, kap, alp):
    """Host-side final reduction (permuted space), mirroring the reference."""
    NC = NORM_CONST

    c1 = G[0][0]                         # sum_i q_i * e1
    M1 = G[1][1:4].T                     # [N,3] sum_i s1 * u_i
    c2 = G[1][4]
    c3 = G[1][0]
    M2 = G[1][5:8].T                     # [N,3] sum_i q_i s1 r_i
    S = G[2]                             # s2*rinv2 contractions  [21, N]
    MS = S[1:4].T
    cS = S[4]
    MS2 = S[8:17]                        # [9, N]  rows 3c+d
    MS3 = S[17:20].T

    e_phi_qq = NC * c1
    e_phi_u = NC * ((rc64 * M1).sum(1) - c2)
    e_phi = e_phi_qq + e_phi_u
    pot = 0.5 * np.dot(e_phi_qq, q64) + np.dot(e_phi_u, q64)

    e_field1 = NC * (rc64 * c3[:, None] - M2)
    c4 = (rc64 * MS).sum(1) - cS
    M3 = np.zeros((N, 3))
    for d in range(3):
        M3[:, d] = sum(rc64[:, c] * MS2[3 * c + d] for c in range(3)) - MS3[:, d]
    T1 = rc64 * c4[:, None] - M3
    E_u = NC * (T1 - M1)
    e_field = e_field1 + E_u

    P2 = (u64 * T1).sum()
    P1 = (u64 * M1).sum()
    pot = pot - 0.5 * NC * (P2 - P1)

    q_ind = -kap * e_phi
    pot = pot + 0.5 * np.dot(e_phi, q_ind)
    u_ind = alp[:, None] * e_field
    pot = pot - 0.5 * (e_field * u_ind).sum()
    return pot, q_ind, u_ind


def _run(in_maps, trace=False, **kw):
    if "nc" not in _CACHE:
        _CACHE["nc"] = _build()
    last_exc = None
    for attempt in range(4):
        try:
            return run_bass_kernel_spmd(_CACHE["nc"], in_maps,
                                        core_ids=list(range(N_CORES)),
                                        trace=trace, **kw)
        except Exception as e:  # transient NRT_EXEC_UNIT_UNRECOVERABLE flakes
            last_exc = e
            time.sleep(3.0 * (attempt + 1))
    raise last_exc


def kernel(q, r, cell, batch, u, kappa, alpha):
    q = np.asarray(q, np.float32)
    r = np.asarray(r, np.float32)
    u = np.asarray(u, np.float32)
    kappa = np.asarray(kappa, np.float32)
    alpha = np.asarray(alpha, np.float32)

    in_maps, V, perm, pairs, rc64 = _prep_inputs(q, r, u)
    res = _run(in_maps)

    # gather: out[jb, w, col, jloc] per core -> G[w, col, j] summed over cores
    G = np.zeros((NSTREAM, NCOL, N), np.float64)
    for c in range(N_CORES):
        o = res.results[c]["out"].astype(np.float64)     # [NJB,3,NCOL,JBLK]
        o = o.transpose(1, 2, 0, 3).reshape(NSTREAM, NCOL, N)
        G += np.roll(o, c * ROWS, axis=-1)

    G = _host_corrections(G, V, pairs, rc64)
    kernel._last_G = G

    q64 = q.astype(np.float64)[perm]
    u64 = u.astype(np.float64)[perm]
    kap = kappa.astype(np.float64)[perm]
    alp = alpha.astype(np.float64)[perm]
    pot, q_ind_p, u_ind_p = _combine(G, q64, rc64, u64, kap, alp)

    q_ind = np.empty(N, np.float64)
    u_ind = np.empty((N, 3), np.float64)
    q_ind[perm] = q_ind_p
    u_ind[perm] = u_ind_p
    return (np.array([pot], np.float32), q_ind.astype(np.float32),
            u_ind.astype(np.float32))


# revision 14
# speedup vs baseline: 3649.2445x; 3649.2445x over previous
"""Ewald realspace (N=3072) on 8 Trainium2 NeuronCores.

Row-sharded: core c owns rows i in [c*384, (c+1)*384) (in a permuted atom
order). Each core computes, for every column j, partial contractions
sum_i W[i,j] * V[i,col] for three pairwise kernels W (e1 = erf*rinv, s1,
SS = s2*rinv2) against a 21-column matrix V of per-atom quantities. The host
combines the 8 partial results into e_phi / e_field / pot exactly as the
reference does.

Device math per 128x512 block (i on partitions, j on free dim):
  d2  = |r_i|^2 + |r_j|^2 - 2 r_i.r_j   (plain-fp32 PE matmul, K=5,
        box-centered coordinates to minimize cancellation error)
        + 2^40 on the diagonal AND on close pairs (DVE add of a host mask)
  rnorm  = sqrt(d2)            [ACT Sqrt;   sqrt set]
  d2sb   = copy(d2)            [ACT Copy;   any set]
  gaussC = exp(-d2/2 + ln(2A/sqrt(pi)))  [ACT Exp;  exp set; 2-ULP table]
  erf_t  = erf(A*rnorm)        [ACT Erf;    sigmoid set]
  rinv   = recip_approx_fast(rnorm)   [DVE]
  rinv2  = Square(rinv)        [ACT Square; any set]
  e1 = erf_t*rinv;  t2 = e1 - gaussC;  s1 = t2*rinv2
  s2 = 3*s1 - gaussC  (pairwise - the cancellation is catastrophic if
                       deferred past the contraction);  SS = s2*rinv2
All three streams are contracted in plain fp32 (PE 2-pass matmuls) with fp32
PSUM accumulation over the core's 3 row-tiles.

Close pairs (d2 < CLOSE_CUT) break the s2 double-cancellation under any
device noise, and the r_j - r_i contraction decomposition amplifies their
error by |r_j|/|r_ij|. So: atoms are permuted so every close pair lands in
one 128-row tile, those pairs are padded out on-device (d2 += 2^40 via the
mask), and the host adds their exact fp64 contributions.

ACT table sets are phase-grouped (sqrt -> exp -> sigmoid) with explicit
scheduling deps so each set loads exactly once per kernel.

Columns are rotated per-core by -c*384 so the diagonal block always lands in
j-block 0 at offset 128*it - keeping the program SPMD-identical across cores.
"""

import time
import numpy as np
from contextlib import ExitStack

import concourse.bass as bass
import concourse.tile as tile
from concourse import bacc, mybir
from concourse.bass_utils import run_bass_kernel_spmd

N = 3072
N_CORES = 8
ROWS = N // N_CORES          # 384
ITILES = ROWS // 128         # 3
JBLK = 512
NJB = N // JBLK              # 6
NCOL = 21                    # V columns actually used
NSTREAM = 3                  # e1, s1, SS

SIGMA = 1.0
NORM_FACTOR = 90.4756
NORM_CONST = NORM_FACTOR / (2.0 * np.pi)
A = 1.0 / (SIGMA * 2.0 ** 0.5)
LN_C2P = float(np.log(2.0 * A / np.sqrt(np.pi)))   # exp bias -> C2P*gauss
BIG = 2.0 ** 40              # d2 pad for diagonal + close pairs
DIAG_E1 = 2.0 ** -20         # e1 value on padded entries (1/sqrt(BIG))
CLOSE_CUT = 2.0              # d2 threshold for host-corrected close pairs

F32 = mybir.dt.float32
AF = mybir.ActivationFunctionType

_CACHE = {}


def _build():
    from concourse.tile_rust import add_dep_helper

    nc = bacc.Bacc("TRN2", target_bir_lowering=False, debug=False,
                   num_devices=N_CORES)
    lhs5_ap = nc.dram_tensor("lhs5", [5, ROWS], F32, kind="ExternalInput").ap()
    rhs5_ap = nc.dram_tensor("rhs5", [5, N], F32, kind="ExternalInput").ap()
    vall_ap = nc.dram_tensor("vall", [128, ITILES * 32], F32,
                             kind="ExternalInput").ap()
    eyek_ap = nc.dram_tensor("eyek", [128, ROWS], F32,
                             kind="ExternalInput").ap()
    out_ap = nc.dram_tensor("out", [NJB, NSTREAM, NCOL, JBLK], F32,
                            kind="ExternalOutput").ap()

    with tile.TileContext(nc) as tc:
        with ExitStack() as ctx:
            const = ctx.enter_context(tc.tile_pool(name="const", bufs=1))
            psum_d2 = ctx.enter_context(
                tc.tile_pool(name="psum_d2", bufs=3, space="PSUM"))
            psum_out = ctx.enter_context(
                tc.tile_pool(name="psum_out", bufs=1, space="PSUM"))
            rnorm_p = ctx.enter_context(tc.tile_pool(name="rnorm", bufs=18))
            d2sb_p = ctx.enter_context(tc.tile_pool(name="d2sb", bufs=18))
            gauss_p = ctx.enter_context(tc.tile_pool(name="gauss", bufs=18))
            trans = ctx.enter_context(tc.tile_pool(name="trans", bufs=3))
            outsb_p = ctx.enter_context(tc.tile_pool(name="outsb", bufs=2))

            lhs5 = const.tile([5, ROWS], F32)
            nc.sync.dma_start(lhs5[:], lhs5_ap[:])
            rhs5 = const.tile([5, N], F32)
            nc.sync.dma_start(rhs5[:], rhs5_ap[:])
            vall_f = const.tile([128, ITILES * 32], F32)
            nc.sync.dma_start(vall_f[:], vall_ap[:])
            eyek = const.tile([128, ROWS], F32)
            nc.sync.dma_start(eyek[:], eyek_ap[:])
            # per-partition bias constant for the Exp activation
            expbias = const.tile([128, 1], F32)
            nc.vector.memset(expbias[:], LN_C2P)

            nb = NJB * ITILES
            rnorm = [None] * nb
            d2sb = [None] * nb
            gauss = [None] * nb

            # ------------- P1: d2 matmuls + Sqrt + Copy (sqrt set) ---------
            last_act = None
            for jb in range(NJB):
                for it in range(ITILES):
                    b = jb * ITILES + it
                    d2 = psum_d2.tile([128, JBLK], F32, tag="d2")
                    nc.tensor.matmul(
                        d2[:],
                        lhs5[:, it * 128:(it + 1) * 128],
                        rhs5[:, jb * JBLK:(jb + 1) * JBLK],
                        start=True, stop=True)
                    if jb == 0:
                        off = it * 128
                        nc.vector.tensor_add(
                            d2[:, off:off + 128], d2[:, off:off + 128],
                            eyek[:, off:off + 128])
                    rn = rnorm_p.tile([128, JBLK], F32, tag="rnorm")
                    nc.scalar.activation(rn[:], d2[:], AF.Sqrt)
                    ds = d2sb_p.tile([128, JBLK], F32, tag="d2sb")
                    i1b = nc.scalar.copy(ds[:], d2[:])
                    last_act = i1b
                    rnorm[b] = rn
                    d2sb[b] = ds

            # ------------- P2: Exp (exp set) -------------------------------
            p1_last = last_act
            for b in range(nb):
                ga = gauss_p.tile([128, JBLK], F32, tag="gauss")
                i2 = nc.scalar.activation(ga[:], d2sb[b][:], AF.Exp,
                                          scale=-0.5, bias=expbias[:])
                add_dep_helper(i2.ins, p1_last.ins, sync=False,
                               reason="ACT phase order: exp after all sqrt")
                last_act = i2
                gauss[b] = ga

            # ------- P3: Erf + Square (sigmoid set) + DVE chain + PE -------
            p2_last = last_act
            for jb in range(NJB):
                outs = [psum_out.tile([NCOL, JBLK], F32, tag=f"out{w}",
                                      name=f"out{w}_jb{jb}")
                        for w in range(NSTREAM)]
                for it in range(ITILES):
                    b = jb * ITILES + it
                    erf_t = trans.tile([128, JBLK], F32, tag="erf")
                    i3 = nc.scalar.activation(erf_t[:], rnorm[b][:], AF.Erf,
                                              scale=float(A))
                    add_dep_helper(i3.ins, p2_last.ins, sync=False,
                                   reason="ACT phase order: erf after all exp")
                    rinv = trans.tile([128, JBLK], F32, tag="rinv")
                    nc.vector.reciprocal_approx_fast(rinv[:], rnorm[b][:])
                    rinv2 = trans.tile([128, JBLK], F32, tag="rinv2")
                    i4 = nc.scalar.activation(rinv2[:], rinv[:], AF.Square)
                    add_dep_helper(i4.ins, p2_last.ins, sync=False,
                                   reason="ACT phase order: square after exp")

                    e1 = trans.tile([128, JBLK], F32, tag="e1")
                    nc.vector.tensor_mul(e1[:], erf_t[:], rinv[:])
                    t2 = trans.tile([128, JBLK], F32, tag="t2")
                    nc.vector.tensor_sub(t2[:], e1[:], gauss[b][:])
                    s1 = trans.tile([128, JBLK], F32, tag="s1")
                    nc.vector.tensor_mul(s1[:], t2[:], rinv2[:])
                    s3 = trans.tile([128, JBLK], F32, tag="s3")
                    nc.vector.tensor_scalar_mul(s3[:], s1[:], 3.0)
                    s2 = trans.tile([128, JBLK], F32, tag="s2")
                    nc.vector.tensor_sub(s2[:], s3[:], gauss[b][:])
                    ss = trans.tile([128, JBLK], F32, tag="ss")
                    nc.vector.tensor_mul(ss[:], s2[:], rinv2[:])

                    for w, st in ((0, e1), (1, s1), (2, ss)):
                        nc.tensor.matmul(
                            outs[w][:],
                            vall_f[:, it * 32: it * 32 + NCOL],
                            st[:],
                            start=(it == 0), stop=(it == ITILES - 1))

                for w in range(NSTREAM):
                    osb = outsb_p.tile([NCOL, JBLK], F32, tag=f"osb{w}",
                                       name=f"osb{w}_jb{jb}")
                    nc.scalar.copy(osb[:], outs[w][:])
                    nc.sync.dma_start(out_ap[jb, w], osb[:])

    nc.compile()
    return nc


def _close_pair_permutation(r64):
    """Find close pairs (d2 < CLOSE_CUT) and a permutation packing each
    connected component of the close-pair graph into one 128-atom tile.

    Returns (perm, pairs) where pairs are ordered (i, j) index pairs IN THE
    PERMUTED numbering, i != j, covering both directions of every close pair
    that ended up intra-tile."""
    d2 = ((r64[:, None, :] - r64[None, :, :]) ** 2).sum(-1)
    np.fill_diagonal(d2, np.inf)
    ii, jj = np.nonzero(d2 < CLOSE_CUT)
    # union-find over close-pair graph
    parent = np.arange(N)

    def find(x):
        while parent[x] != x:
            parent[x] = parent[parent[x]]
            x = parent[x]
        return x

    for i, j in zip(ii, jj):
        ri, rj = find(i), find(j)
        if ri != rj:
            parent[ri] = rj
    comps = {}
    for i in range(N):
        comps.setdefault(find(i), []).append(i)
    groups = sorted(comps.values(), key=len, reverse=True)
    # first-fit pack into 24 tiles of 128
    ntiles = N // 128
    bins = [[] for _ in range(ntiles)]
    leftovers = []
    for g in groups:
        if len(g) > 128:
            leftovers.extend(g)     # unpackable; pairs inside stay uncorrected
            continue
        placed = False
        for b in bins:
            if len(b) + len(g) <= 128:
                b.extend(g)
                placed = True
                break
        if not placed:
            leftovers.extend(g)
    for x in leftovers:
        for b in bins:
            if len(b) < 128:
                b.append(x)
                break
    perm = np.array([x for b in bins for x in b])
    assert len(perm) == N and len(set(perm.tolist())) == N
    inv = np.empty(N, np.int64)
    inv[perm] = np.arange(N)
    # close pairs in permuted numbering, keep only intra-tile ones
    pi, pj = inv[ii], inv[jj]
    intra = (pi // 128) == (pj // 128)
    pairs = np.stack([pi[intra], pj[intra]], 1)
    return perm, pairs


def _prep_inputs(q, r, u):
    """Permute atoms, center coordinates, build per-core input maps.

    Returns (in_maps, V, perm, pairs, rc64) - everything in PERMUTED space."""
    r64g = r.astype(np.float64)
    perm, pairs = _close_pair_permutation(r64g)
    center = (r64g.max(0) + r64g.min(0)) / 2.0
    rc64 = (r64g - center)[perm]
    q64 = q.astype(np.float64)[perm]
    u64 = u.astype(np.float64)[perm]
    rr = (rc64 ** 2).sum(1)
    w = (u64 * rc64).sum(1)

    rhs5_g = np.stack([rc64[:, 0], rc64[:, 1], rc64[:, 2],
                       np.ones(N), rr]).astype(np.float32)        # [5, N]
    lhs5_g = np.stack([-2 * rc64[:, 0], -2 * rc64[:, 1], -2 * rc64[:, 2],
                       rr, np.ones(N)]).astype(np.float32)        # [5, N]

    V = np.zeros((N, NCOL), np.float64)
    V[:, 0] = q64
    V[:, 1:4] = u64
    V[:, 4] = w
    V[:, 5:8] = q64[:, None] * rc64
    for c in range(3):
        for d in range(3):
            V[:, 8 + 3 * c + d] = u64[:, c] * rc64[:, d]
    V[:, 17:20] = w[:, None] * rc64
    V[:, 20] = 1.0
    Vf = V.astype(np.float32)

    # pad mask per tile: diagonal + intra-tile close pairs
    eyek_g = np.zeros((N // 128, 128, 128), np.float32)
    for t in range(N // 128):
        eyek_g[t][np.arange(128), np.arange(128)] = BIG
    for i, j in pairs:
        t = i // 128
        eyek_g[t][i % 128, j % 128] = BIG

    in_maps = []
    for c in range(N_CORES):
        sl = slice(c * ROWS, (c + 1) * ROWS)
        vall = np.zeros((128, ITILES * 32), np.float32)
        eyek = np.zeros((128, ROWS), np.float32)
        for it in range(ITILES):
            gt = c * ITILES + it
            rows = slice(gt * 128, (gt + 1) * 128)
            vall[:, it * 32: it * 32 + NCOL] = Vf[rows]
            eyek[:, it * 128:(it + 1) * 128] = eyek_g[gt]
        in_maps.append({
            "lhs5": np.ascontiguousarray(lhs5_g[:, sl]),
            "rhs5": np.ascontiguousarray(np.roll(rhs5_g, -c * ROWS, axis=1)),
            "vall": vall,
            "eyek": eyek,
        })
    return in_maps, V, perm, pairs, rc64


def _host_corrections(G, V, pairs, rc64):
    """Remove padded-entry spurious e1 and add exact fp64 close-pair terms."""
    # spurious e1 = DIAG_E1 on the diagonal and every padded pair
    G[0] -= V.T * DIAG_E1
    if len(pairs):
        np.subtract.at(G[0].T, pairs[:, 1], V[pairs[:, 0]] * DIAG_E1)
        d2c = ((rc64[pairs[:, 0]] - rc64[pairs[:, 1]]) ** 2).sum(1)
        rn = np.sqrt(d2c)
        ri = 1.0 / rn
        ri2 = ri * ri
        from scipy.special import erf as serf
        ef = serf(A * rn)
        ga = (2 * A / np.sqrt(np.pi)) * np.exp(-0.5 * d2c)
        e1c = ef * ri
        s1c = (e1c - ga) * ri2
        SSc = (3 * s1c - ga) * ri2
        np.add.at(G[0].T, pairs[:, 1], V[pairs[:, 0]] * e1c[:, None])
        np.add.at(G[1].T, pairs[:, 1], V[pairs[:, 0]] * s1c[:, None])
        np.add.at(G[2].T, pairs[:, 1], V[pairs[:, 0]] * SSc[:, None])
    return G


def _combine(G, q64, rc64, u64, kap, alp):
    """Host-side final reduction (permuted space), mirroring the reference."""
    NC = NORM_CONST

    c1 = G[0][0]                         # sum_i q_i * e1
    M1 = G[1][1:4].T                     # [N,3] sum_i s1 * u_i
    c2 = G[1][4]
    c3 = G[1][0]
    M2 = G[1][5:8].T                     # [N,3] sum_i q_i s1 r_i
    S = G[2]                             # s2*rinv2 contractions  [21, N]
    MS = S[1:4].T
    cS = S[4]
    MS2 = S[8:17]                        # [9, N]  rows 3c+d
    MS3 = S[17:20].T

    e_phi_qq = NC * c1
    e_phi_u = NC * ((rc64 * M1).sum(1) - c2)
    e_phi = e_phi_qq + e_phi_u
    pot = 0.5 * np.dot(e_phi_qq, q64) + np.dot(e_phi_u, q64)

    e_field1 = NC * (rc64 * c3[:, None] - M2)
    c4 = (rc64 * MS).sum(1) - cS
    M3 = np.zeros((N, 3))
    for d in range(3):
        M3[:, d] = sum(rc64[:, c] * MS2[3 * c + d] for c in range(3)) - MS3[:, d]
    T1 = rc64 * c4[:, None] - M3
    E_u = NC * (T1 - M1)
    e_field = e_field1 + E_u

    P2 = (u64 * T1).sum()
    P1 = (u64 * M1).sum()
    pot = pot - 0.5 * NC * (P2 - P1)

    q_ind = -kap * e_phi
    pot = pot + 0.5 * np.dot(e_phi, q_ind)
    u_ind = alp[:, None] * e_field
    pot = pot - 0.5 * (e_field * u_ind).sum()
    return pot, q_ind, u_ind


def _run(in_maps, trace=False, reps=1, **kw):
    key = f"nc{reps}"
    if key not in _CACHE:
        _CACHE[key] = _build(reps)
    last_exc = None
    for attempt in range(7):
        try:
            return run_bass_kernel_spmd(_CACHE[key], in_maps,
                                        core_ids=list(range(N_CORES)),
                                        trace=trace, **kw)
        except Exception as e:  # transient NRT_EXEC_UNIT_UNRECOVERABLE flakes
            last_exc = e
            time.sleep(5.0 * (attempt + 1))
    raise last_exc


def kernel(q, r, cell, batch, u, kappa, alpha):
    q = np.asarray(q, np.float32)
    r = np.asarray(r, np.float32)
    u = np.asarray(u, np.float32)
    kappa = np.asarray(kappa, np.float32)
    alpha = np.asarray(alpha, np.float32)

    in_maps, V, perm, pairs, rc64 = _prep_inputs(q, r, u)
    res = _run(in_maps)

    # gather: out[jb, w, col, jloc] per core -> G[w, col, j] summed over cores
    G = np.zeros((NSTREAM, NCOL, N), np.float64)
    for c in range(N_CORES):
        o = res.results[c]["out"].astype(np.float64)     # [NJB,3,NCOL,JBLK]
        o = o.transpose(1, 2, 0, 3).reshape(NSTREAM, NCOL, N)
        G += np.roll(o, c * ROWS, axis=-1)

    G = _host_corrections(G, V, pairs, rc64)
    kernel._last_G = G

    q64 = q.astype(np.float64)[perm]
    u64 = u.astype(np.float64)[perm]
    kap = kappa.astype(np.float64)[perm]
    alp = alpha.astype(np.float64)[perm]
    pot, q_ind_p, u_ind_p = _combine(G, q64, rc64, u64, kap, alp)

    q_ind = np.empty(N, np.float64)
    u_ind = np.empty((N, 3), np.float64)
    q_ind[perm] = q_ind_p
    u_ind[perm] = u_ind_p
    return (np.array([pot], np.float32), q_ind.astype(np.float32),
            u_ind.astype(np.float32))
